# revision 2
# baseline (speedup 1.0000x reference)
"""Trainium2 Bass kernel v3: 16-head cross attention, tensor-parallel over 8 cores.

v1 baseline structure (bf16, transposed scores, ones-column softmax sums,
per-head score matmuls auto-row-tiled to PE quadrants) with:
  - v2's startup DMA pattern (3-queue interleave, block-0 slices first):
    first exp at ~8us instead of ~30us.
  - Scalar engine reserved for the exp stream in steady state: no scalar
    DMA issues or copies between the first and last ACT (the kernel is
    exp-bound at ~18us/block; every non-ACT scalar-queue entry stalls it).
  - qT(2)/qT(3) emission moved out of block 0 into blocks 1/2 (PE slack
    there; shrinks the PE-bound block 0).
  - Tail rewritten with a deep psum rotation (ps_sc 2-slab tiles + ps_pv)
    and copies alternating Scalar/DVE: ~18us -> ~6us.
"""

import os
import sys

for _p in ("/opt/trn_rl_repo", "/root/.axon_site/_ro/trn_rl_repo"):
    if os.path.isdir(_p) and _p not in sys.path:
        sys.path.insert(0, _p)

import numpy as np
import ml_dtypes

import concourse.bass as bass
import concourse.mybir as mybir
import concourse.tile as tile
from concourse import bacc
from concourse.bass_utils import run_bass_kernel_spmd

P = 128
N_TOK = 2048
M_TOK = 2048
D = 1024
C = 768
HEADS = 16
DH = 64
NB = 512
DK = D // P  # 8
CK = C // P  # 6
MT = M_TOK // P  # 16
NBLK = N_TOK // NB  # 4
VW = 128
SCALE = 8.0  # sqrt(DH)
DTYPE_MODE = "bf16"

f32 = mybir.dt.float32
bf16 = mybir.dt.bfloat16
Exp = mybir.ActivationFunctionType.Exp


def build_core_program():
    nc = bacc.Bacc("TRN2", target_bir_lowering=False, debug=False)

    xT = nc.declare_dram_parameter("xT", [D, N_TOK], bf16, isOutput=False)
    ctxT = nc.declare_dram_parameter("ctxT", [C, M_TOK], bf16, isOutput=False)
    wq = nc.declare_dram_parameter("wq", [D, P], bf16, isOutput=False)
    wk = nc.declare_dram_parameter("wk", [C, P], bf16, isOutput=False)
    wv = nc.declare_dram_parameter("wv", [C, P], bf16, isOutput=False)
    wo = nc.declare_dram_parameter("wo", [P, D], bf16, isOutput=False)
    yT = nc.declare_dram_parameter("yT", [D, N_TOK], f32, isOutput=True)

    with tile.TileContext(nc) as tc:
        with (
            tc.tile_pool(name="wts", bufs=1) as wts,
            tc.tile_pool(name="att", bufs=4) as att,
            tc.tile_pool(name="yout", bufs=4) as yout,
            tc.tile_pool(name="small", bufs=4) as small,
            tc.tile_pool(name="ps_sc", bufs=2, space="PSUM") as ps_sc,  # 2x2 banks
            tc.tile_pool(name="ps_pv", bufs=3, space="PSUM") as ps_pv,  # 3x1
            tc.tile_pool(name="ps_y", bufs=1, space="PSUM") as ps_y,  # 1x1
        ):
            # ---- ACT exp-table preload under the input DMA ----
            warm = small.tile([1, 8], f32, tag="warm")
            nc.vector.memset(warm[:], 0.0)
            nc.scalar.activation(warm[:], warm[:], Exp)

            # ---- input DMA. Block-0 slices go first, 256-col grain for the
            # ctx side (transfer time halves; each DMA rides one engine at
            # ~22.5 GB/s), spread over all 5 queues so descriptor issue
            # (~0.6us each) doesn't serialize. Later slices use the sync /
            # gpsimd queues only, leaving scalar for the exp stream. ----
            wk_sb = wts.tile([P, CK, P], bf16)
            nc.sync.dma_start(wk_sb[:], wk.ap().rearrange("(p o) e -> p o e", o=CK))
            wq_sb = wts.tile([P, DK, P], bf16)
            nc.scalar.dma_start(wq_sb[:], wq.ap().rearrange("(p o) e -> p o e", o=DK))
            wv_sb = wts.tile([P, CK, P], bf16)
            nc.gpsimd.dma_start(wv_sb[:], wv.ap().rearrange("(p o) e -> p o e", o=CK))
            ctxT_sb = wts.tile([P, CK, M_TOK], bf16)
            xT_sb = wts.tile([P, DK, N_TOK], bf16)

            q3 = (nc.sync, nc.scalar, nc.gpsimd)
            qi = 0

            def _b0_dma(dst, src):
                nonlocal qi
                q3[qi % 3].dma_start(dst, src)
                qi += 1

            HB = NB // 2
            for ck in range(CK):
                for h in range(2):
                    _b0_dma(
                        ctxT_sb[:, ck, h * HB : (h + 1) * HB],
                        ctxT.ap()[ck * P : (ck + 1) * P, h * HB : (h + 1) * HB],
                    )
            for dk in range(DK):
                _b0_dma(xT_sb[:, dk, :NB], xT.ap()[dk * P : (dk + 1) * P, :NB])

            # PE warm-up: two fp32 dummy matmuls (~1.7us each even cold) keep
            # the PE busy through the DMA wait so the HAM clock-gate opens
            # before the first real matmuls.
            warmpe = small.tile([P, NB], f32, tag="warmpe")
            nc.vector.memset(warmpe[:], 0.0)
            pwu = ps_y.tile([P, NB], f32, tag="ps_y", name="ps_warm")
            for _ in range(2):
                nc.tensor.matmul(
                    pwu[:], warmpe[:, :P], warmpe[:], start=True, stop=True
                )

            # Late input slices are NOT issued here: issuing them now would
            # put their transfers in flight alongside the block-0 slices and
            # split the DMA-engine bandwidth 5 ways (the first exp is input-
            # bandwidth-bound). They are dribbled out inside block 0's mt
            # loop instead (see late_work below), ordered by first use.
            wo_sb = wts.tile([P, D], bf16)

            late_work = []  # ordered by first use (kT(b) emitted at mt 3b)
            for ck in range(CK):  # ctx block 1: kT(1) at mt 3
                late_work.append(
                    (ctxT_sb[:, ck, NB : 2 * NB],
                     ctxT.ap()[ck * P : (ck + 1) * P, NB : 2 * NB])
                )
            for ck in range(CK):  # ctx block 2: kT(2) at mt 6
                late_work.append(
                    (ctxT_sb[:, ck, 2 * NB : 3 * NB],
                     ctxT.ap()[ck * P : (ck + 1) * P, 2 * NB : 3 * NB])
                )
            for dk in range(DK):  # x block 1 (512-wide): qT(1) at mt 11
                late_work.append(
                    (xT_sb[:, dk, NB : 2 * NB],
                     xT.ap()[dk * P : (dk + 1) * P, NB : 2 * NB])
                )
            late_work.append((wo_sb[:], wo.ap()))
            for ck in range(CK):  # ctx block 3: kT(3) at mt 10
                late_work.append(
                    (ctxT_sb[:, ck, 3 * NB :],
                     ctxT.ap()[ck * P : (ck + 1) * P, 3 * NB :])
                )
            for dk in range(DK):  # x blocks 2-3: needed in blocks 1-2
                late_work.append(
                    (xT_sb[:, dk, 2 * NB :],
                     xT.ap()[dk * P : (dk + 1) * P, 2 * NB :])
                )

            def emit_late(n):
                nonlocal qi
                for _ in range(n):
                    if not late_work:
                        return
                    dst, src = late_work.pop(0)
                    (nc.sync, nc.gpsimd)[qi % 2].dma_start(dst, src)
                    qi += 1

            # ---- persistent intermediates ----
            kT_sb = wts.tile([P, M_TOK], bf16)  # [dh(2 heads), m]
            qT_sb = wts.tile([P, N_TOK], bf16)  # [dq(2 heads), n]
            vA_sb = wts.tile([P, MT, VW], bf16)  # col0=ones, cols 64..127 = v
            vB_sb = wts.tile([P, MT, VW], bf16)
            oT_sb = wts.tile([P, N_TOK], bf16)  # attn out^T, both heads

            # cols 64..127 are always written by emit_v before PV reads them;
            # only the ones column and the padding cols 1..63 need init.
            nc.vector.memset(vA_sb[:, :, 0:DH], 0.0)
            nc.vector.memset(vB_sb[:, :, 0:DH], 0.0)
            nc.vector.memset(vA_sb[:, :, 0:1], 1.0)
            nc.vector.memset(vB_sb[:, :, 0:1], 1.0)

            def emit_kT(nb, pool):
                if pool is ps_sc:
                    ps = ps_sc.tile([P, 2, NB], f32, tag="ps_sc", name="ps_k")
                    pk = ps[:, 0, :]
                else:
                    pk = pool.tile([P, NB], f32, tag="ps_y", name="ps_ky")
                for ck in range(CK):
                    nc.tensor.matmul(
                        pk,
                        wk_sb[:, ck, :],
                        ctxT_sb[:, ck, nb * NB : (nb + 1) * NB],
                        start=(ck == 0),
                        stop=(ck == CK - 1),
                    )
                nc.vector.tensor_copy(kT_sb[:, nb * NB : (nb + 1) * NB], pk)

            def emit_qT(nb, pool):
                if pool is ps_sc:
                    ps = ps_sc.tile([P, 2, NB], f32, tag="ps_sc", name="ps_q")
                    pq = ps[:, 0, :]
                else:
                    pq = pool.tile([P, NB], f32, tag="ps_y", name="ps_qy")
                for dk in range(DK):
                    nc.tensor.matmul(
                        pq,
                        wq_sb[:, dk, :],
                        xT_sb[:, dk, nb * NB : (nb + 1) * NB],
                        start=(dk == 0),
                        stop=(dk == DK - 1),
                    )
                nc.vector.tensor_copy(qT_sb[:, nb * NB : (nb + 1) * NB], pq)

            def emit_v(mt):
                ps = ps_pv.tile([P, NB], f32, tag="ps_pv", name="ps_v")
                pv = ps[:, :P]
                for ck in range(CK):
                    nc.tensor.matmul(
                        pv,
                        ctxT_sb[:, ck, mt * P : (mt + 1) * P],
                        wv_sb[:, ck, :],
                        start=(ck == 0),
                        stop=(ck == CK - 1),
                    )
                nc.vector.tensor_copy(vA_sb[:, mt, DH:VW], pv[:, :DH])
                nc.vector.tensor_copy(vB_sb[:, mt, DH:VW], pv[:, DH:P])

            with nc.named_scope("ph1_kT"):
                emit_kT(0, ps_sc)
            with nc.named_scope("ph1_qT"):
                emit_qT(0, ps_sc)

            # ---- phase 2 helpers ----
            def emit_norm(pvA, pvB, nsl):
                for h, pv in ((0, pvA), (1, pvB)):
                    rcf = small.tile([1, NB], f32, tag="recip_f32")
                    nc.vector.reciprocal_approx_fast(rcf[:], pv[0:1, :])
                    bcs = small.tile([DH, NB], f32, tag="bcast_sb")
                    nc.gpsimd.partition_broadcast(bcs[:], rcf[:])
                    nc.vector.tensor_mul(
                        oT_sb[h * DH : (h + 1) * DH, nsl], pv[DH:P, :], bcs[:]
                    )

            def emit_proj_step(dt_i, nsl, tail=False):
                # in-block steps: single ps_y bank, DVE copy (scalar is the
                # exp engine). Tail: rotate ps_sc 2-slab tiles + ps_pv banks
                # (4-deep), copies alternate Scalar/DVE.
                if tail:
                    ph = dt_i % 4
                    if ph in (0, 1):
                        if ph == 0:
                            emit_proj_step.t2 = ps_sc.tile(
                                [P, 2, NB], f32, tag="ps_sc", name="py2"
                            )
                        py = emit_proj_step.t2[:, ph, :]
                    else:
                        py = ps_pv.tile([P, NB], f32, tag="ps_pv", name="pyv")
                else:
                    py = ps_y.tile([P, NB], f32, tag="ps_y")
                nc.tensor.matmul(
                    py[:],
                    wo_sb[:, dt_i * P : (dt_i + 1) * P],
                    oT_sb[:, nsl],
                    start=True,
                    stop=True,
                )
                ys = yout.tile([P, NB], f32, tag="yout")
                if tail and dt_i % 2 == 1:
                    nc.scalar.copy(ys[:], py[:])
                else:
                    nc.vector.tensor_copy(ys[:], py[:])
                eng = nc.sync if dt_i % 2 == 0 else nc.gpsimd
                eng.dma_start(yT.ap()[dt_i * P : (dt_i + 1) * P, nsl], ys[:])

            # ---- phase 2: attention ----
            prev = None
            for nb in range(NBLK):
                nsl = slice(nb * NB, (nb + 1) * NB)
                with nc.named_scope(f"ph2_att{nb}"):
                    pvA = ps_pv.tile([P, NB], f32, tag="ps_pv")
                    pvB = ps_pv.tile([P, NB], f32, tag="ps_pv")
                    for mt in range(MT):
                        msl = slice(mt * P, (mt + 1) * P)
                        if nb == 0:
                            # v chunks + kT m-blocks are needed inside block 0
                            # itself (scores sweep all m); qT(1) by block 1.
                            emit_late(3)
                            emit_v(mt)
                            if mt in (3, 6, 10):
                                emit_kT(1 + (3, 6, 10).index(mt), pool=ps_y)
                            elif mt == 11:
                                emit_qT(1, pool=ps_y)
                        elif nb in (1, 2) and mt == 11:
                            emit_qT(nb + 1, pool=ps_y)
                        # scores for both heads: one 2-bank psum tile; the two
                        # matmuls run on PE row-quadrants (0,0)/(64,0)
                        sc = ps_sc.tile([P, 2, NB], f32, tag="ps_sc")
                        nc.tensor.matmul(
                            sc[:, 0, :], kT_sb[0:DH, msl], qT_sb[0:DH, nsl],
                            start=True, stop=True,
                        )
                        nc.tensor.matmul(
                            sc[:, 1, :], kT_sb[DH:P, msl], qT_sb[DH:P, nsl],
                            start=True, stop=True,
                        )
                        # exp of both heads in one ACT op
                        at = att.tile([P, 2, NB], bf16, tag="att")
                        nc.scalar.activation(at[:], sc[:], Exp)
                        # PV accumulation (ones column gives softmax sums)
                        nc.tensor.matmul(
                            pvA[:VW, :],
                            vA_sb[:, mt, :],
                            at[:, 0, :],
                            start=(mt == 0),
                            stop=(mt == MT - 1),
                        )
                        nc.tensor.matmul(
                            pvB[:VW, :],
                            vB_sb[:, mt, :],
                            at[:, 1, :],
                            start=(mt == 0),
                            stop=(mt == MT - 1),
                        )
                        if prev is not None and 2 <= mt < 10:
                            emit_proj_step(mt - 2, prev[2])
                    emit_norm(pvA, pvB, nsl)
                prev = (pvA, pvB, nsl)

            with nc.named_scope("ph2_tail"):
                for dt_i in range(8):
                    emit_proj_step(dt_i, prev[2], tail=True)

    nc.compile()
    return nc


_NC_CACHE = {}


def _get_nc():
    if "v5" not in _NC_CACHE:
        _NC_CACHE["v5"] = build_core_program()
    return _NC_CACHE["v5"]


def _shuffle_w(w):
    # [o*P + p, e] -> [p*o_n + o, e]: contiguous [P, o, e] DMA tile
    o_n = w.shape[0] // P
    return np.ascontiguousarray(
        w.reshape(o_n, P, w.shape[1]).transpose(1, 0, 2).reshape(w.shape)
    )


def _prep_in_maps(x, ctx, Wq, Wk, Wv, Wo):
    bf = ml_dtypes.bfloat16
    xT = np.ascontiguousarray(x.T).astype(bf)
    ctxT = np.ascontiguousarray(ctx.T).astype(bf)
    Wq_s = (Wq / SCALE).astype(np.float32)
    in_maps = []
    for cc in range(8):
        csl = slice(cc * P, (cc + 1) * P)
        in_maps.append(
            {
                "xT": xT,
                "ctxT": ctxT,
                "wq": _shuffle_w(np.ascontiguousarray(Wq_s[:, csl])).astype(bf),
                "wk": _shuffle_w(np.ascontiguousarray(Wk[:, csl])).astype(bf),
                "wv": _shuffle_w(np.ascontiguousarray(Wv[:, csl])).astype(bf),
                "wo": np.ascontiguousarray(Wo[csl, :]).astype(bf),
            }
        )
    return in_maps


def run(x, ctx, Wq, Wk, Wv, Wo, trace=False):
    nc = _get_nc()
    in_maps = _prep_in_maps(x, ctx, Wq, Wk, Wv, Wo)
    res = run_bass_kernel_spmd(nc, in_maps, core_ids=list(range(8)), trace=trace)
    acc = np.zeros((D, N_TOK), np.float32)
    for r in res.results:
        acc += r["yT"]
    return np.ascontiguousarray(acc.T), res


def kernel(x, ctx, Wq, Wk, Wv, Wo):
    x = np.asarray(x, dtype=np.float32)
    ctx = np.asarray(ctx, dtype=np.float32)
    Wq = np.asarray(Wq, dtype=np.float32)
    Wk = np.asarray(Wk, dtype=np.float32)
    Wv = np.asarray(Wv, dtype=np.float32)
    Wo = np.asarray(Wo, dtype=np.float32)
    y, _ = run(x, ctx, Wq, Wk, Wv, Wo, trace=False)
    return y
